# revision 2
# baseline (speedup 1.0000x reference)
"""Trainium2 Bass kernel for a GNN message-passing layer — gathered edition.

Math (reference):
  h1[i,j,:] = concat(x_i, x_j, ef_ij) @ W1 + b1              (pre-relu hidden)
  msg       = relu(h1) @ W2 + b2
  agg[i]    = sum_{j: adj>0} msg[i,j] / max(deg,1),  deg = sum_j adj[i,j]
  out       = relu(concat(x, agg) @ U1 + ub1) @ U2 + ub2

Restructure: @W2 commutes with the masked sum, so only
  S[i] = sum_{j in nbr(i)} relu(h1[i,j,:])
is needed on-device.  h1 = ef_ij@W1e + x_j@W1j + (x_i@W1i + b1) = C + B_j + a_i.

Design: the host GATHERS each row's actual neighbors (~512 of 1024) into a
packed [96, sum_J] fp8/bf16 tensor (rows: efT_i0(16) efT_i1(16) xT_i0(32)
xT_i1(32) per i-pair column block).  Rows are degree-sorted and paired so a
pair's column count J_p = max(cnt_i0, cnt_i1); columns the shorter row leaves
empty are zeros, whose exact relu(a)/relu(-a) contribution — and the whole
"+fixup, ×1/deg" epilogue linear algebra — is folded into host-precomputed
constants (b2t absorbs W2.T @ (fixup*rdeg); rdeg is applied by the sst
rearrange muls).  The schedule (J_p, offsets, ACT/DVE assignment) is shared
across the 8 cores (max-over-cores at each sorted rank) so one SPMD program
serves all.

Per pair: 1-2 matmuls (K=96, <=512 cols each into one PSUM bank), then ONE
bank-spanning fused relu+bias+reduce (ACT: activation(Relu, bias=a,
accum_out); DVE: tensor_scalar(max, -a, accum_out); greedy cost-balanced
split across both engines).  The pack stays SBUF-resident, fed by chunked
HWDGE DMAs sized so compute starts ~2us in.
"""

import os
import numpy as np
import ml_dtypes
from contextlib import ExitStack

import concourse.bass as bass
import concourse.tile as tile
from concourse import bacc, mybir
from concourse.bass_utils import run_bass_kernel_spmd

N_CORES = 8
N, D, E, H = 1024, 32, 16, 64
RPC = N // N_CORES          # 128 source rows (i) per core
NPAIR = RPC // 2            # 64 i-pairs per core
KTOT = 96                   # efT(2*16) + xT(2*32) moving rows
CHUNKS = [4, 4, 8, 8, 8, 8, 12, 12]   # pairs per pack-DMA chunk
NCHUNK = len(CHUNKS)
CHUNK_B = np.concatenate([[0], np.cumsum(CHUNKS)])   # pair boundaries
BF16 = ml_dtypes.bfloat16

# fp8 e3m4 pack: values are N(0,1) (max ~4.7 << 15.5 range), 4 mantissa
# bits -> ~1.8% rms elementwise, averaged out over ~512-term sums.
PACK_FP8 = os.environ.get("PACK_FP8", "1") == "1"
PACK_NP = ml_dtypes.float8_e3m4 if PACK_FP8 else BF16

# cost-model ns per op: engine_time = J*0.8333 + fixed  (ACT adds accum read)
ACT_FIXED = float(os.environ.get("V4_ACT_FIXED", "370"))
DVE_FIXED = 233.0
CYC = 0.8333
SCR_BUFS = int(os.environ.get("V4_SCR_BUFS", "2"))
UNROLL = int(os.environ.get("V4_UNROLL", "2"))      # phases per For_i iter
STAGGER = os.environ.get("V4_STAGGER", "0") == "1"
INTERLEAVE = os.environ.get("V4_INTERLEAVE", "0") == "1"
INPLACE = os.environ.get("V4_INPLACE", "0") == "1"

_cache = {}


def _mk_schedule(adjacency):
    """Common cross-core schedule from the adjacency: per-core degree-sorted
    pairing, J[p] = max over cores, greedy ACT/DVE balance."""
    adj = np.asarray(adjacency)
    mask = adj > 0
    cnt = mask.sum(1)                      # gathered-edge count per row
    perms = []
    Jc = np.zeros((N_CORES, NPAIR), np.int64)
    for c in range(N_CORES):
        cc = cnt[c * RPC : (c + 1) * RPC]
        perm = np.argsort(cc, kind="stable")
        perms.append(perm)
        Jc[c] = np.maximum(cc[perm[0::2]], cc[perm[1::2]])
    J = np.maximum(Jc.max(0), 8)   # floor guards degenerate all-isolated ranks
    # greedy assignment, widest first, to the engine with less modeled time
    order = np.argsort(-J, kind="stable")
    isact = np.zeros(NPAIR, bool)
    ta = td = 0.0
    for p in order:
        ca = J[p] * CYC + ACT_FIXED
        cd = J[p] * CYC + DVE_FIXED
        if ta + ca <= td + cd:
            isact[p] = True
            ta += ca
        else:
            td += cd
    return tuple(int(x) for x in J), tuple(bool(b) for b in isact), perms


def _build(reps: int = 1):
    J, isact = _cache["sched"]
    offs = np.zeros(NPAIR + 1, np.int64)
    offs[1:] = np.cumsum(J)
    cw = [int(offs[CHUNK_B[q + 1]] - offs[CHUNK_B[q]]) for q in range(NCHUNK)]

    nc = bacc.Bacc(
        "TRN2", target_bir_lowering=False, debug=False, num_devices=N_CORES
    )
    f32 = mybir.dt.float32
    bf = mybir.dt.bfloat16
    pdt = mybir.dt.float8e3 if PACK_FP8 else bf

    t = {}
    def inp(name, shape, dt):
        t[name] = nc.dram_tensor(name, list(shape), dt, kind="ExternalInput").ap()

    for q in range(NCHUNK):
        inp(f"pack{q}", (KTOT, cw[q]), pdt)
    inp("statw", (KTOT, 128), bf)
    inp("c128", (128, 3 * NPAIR), f32)   # abias | nabias | rd128
    # c64: b2t' | w2 | u2 | iden | ub1 | ub2
    C64W = RPC + 3 * H + 2
    inp("c64", (H, C64W), f32)
    inp("u1m", (D + H, H), f32)
    inp("xct", (D, RPC), f32)
    out = nc.dram_tensor("out", [RPC, H], f32, kind="ExternalOutput").ap()

    relu = mybir.ActivationFunctionType.Relu

    with tile.TileContext(nc) as tc:
        with ExitStack() as ctx:
            const = ctx.enter_context(tc.tile_pool(name="const", bufs=1))
            psum = ctx.enter_context(tc.tile_pool(name="psum", bufs=4, space="PSUM"))
            scr = ctx.enter_context(tc.tile_pool(name="scr", bufs=1))
            scrA = ctx.enter_context(tc.tile_pool(name="scrA", bufs=SCR_BUFS))
            scrD = ctx.enter_context(tc.tile_pool(name="scrD", bufs=SCR_BUFS))

            def load_const(name, shape, dt, eng=None):
                sb = const.tile(list(shape), dt, tag=name)
                (eng or nc.sync).dma_start(sb[:], t[name][:])
                return sb

            # prereqs of the first compute go first on both rings:
            # sync: c128+statw (tiny), then odd pack chunks, then epilogue
            # consts; scalar: even pack chunks from the start.
            statw_sb = load_const("statw", (KTOT, 128), bf)
            c128_sb = load_const("c128", (128, 3 * NPAIR), f32)
            abias_sb = c128_sb[:, 0:NPAIR]
            nabias_sb = c128_sb[:, NPAIR : 2 * NPAIR]
            rd128_sb = c128_sb[:, 2 * NPAIR : 3 * NPAIR]

            pk = []
            for q in range(NCHUNK):
                sb = const.tile([KTOT, cw[q]], pdt, tag=f"pack{q}")
                eng = nc.scalar if q % 2 == 0 else nc.sync
                eng.dma_start(sb[:], t[f"pack{q}"][:])
                pk.append(sb)

            # epilogue-only constants load after the pack chunks
            c64_sb = load_const("c64", (H, C64W), f32)
            u1_sb = load_const("u1m", (D + H, H), f32)
            b2t_sb = c64_sb[:, 0:RPC]
            w2_sb = c64_sb[:, RPC : RPC + H]
            u2_sb = c64_sb[:, RPC + H : RPC + 2 * H]
            iden_sb = c64_sb[:, RPC + 2 * H : RPC + 3 * H]
            ub1_sb = c64_sb[:, RPC + 3 * H : RPC + 3 * H + 1]
            ub2_sb = c64_sb[:, RPC + 3 * H + 1 : RPC + 3 * H + 2]

            # combined^T rows: [aggregated (H); x (D)] — agg first so the
            # engine write below starts at partition 0 (HW quadrant rule).
            combt = const.tile([H + D, RPC], f32, tag="combt")
            nc.sync.dma_start(combt[H : H + D, :], t["xct"][:])

            # double-buffered accumulators: phase X's pairs write accs[X]
            # while the interleaved epilogue of the other phase reads accs[1-X]
            accs = []
            for ph in range(2):
                aa = const.tile([128, NPAIR], f32, tag=f"acc_act{ph}")
                ad = const.tile([128, NPAIR], f32, tag=f"acc_dve{ph}")
                nc.vector.memset(aa[:], 0.0)
                nc.vector.memset(ad[:], 0.0)
                accs.append((aa, ad))

            # tiny warmup activation: forces the ACT function-table load
            # (~2.7us) to happen at kernel start, overlapped with input DMAs
            warm = scr.tile([1, 1], f32, tag="warm")
            nc.vector.memset(warm[:], 0.0)
            warmo = scr.tile([1, 1], f32, tag="warmo")
            nc.scalar.activation(warmo[:], warm[:], relu)

            pair_chunk = np.searchsorted(CHUNK_B, np.arange(NPAIR), side="right") - 1

            def _epilogue_stages(ph):
                """Epilogue of phase `ph` as 5 stages of closures; cheap DVE
                ops moved to the otherwise-idle GPSIMD, PSUM copies spread
                over ACT/DVE.  rdeg applied by the (h, s) rearrange muls;
                fixup*rdeg@W2 pre-folded into b2t host-side."""
                aa, ad = accs[ph]
                st = {}

                def s0():
                    t3 = scr.tile([128, NPAIR], f32, tag="t3")
                    nc.gpsimd.tensor_add(t3[:], aa[:], ad[:])
                    sst = scr.tile([H, NPAIR, 2], f32, tag="sst")
                    nc.gpsimd.tensor_mul(sst[:, :, 0], t3[0:H, :], rd128_sb[0:H, :])
                    nc.gpsimd.tensor_mul(sst[:, :, 1], t3[H:128, :], rd128_sb[H:128, :])
                    st["sst"] = sst

                def s1():
                    agp = psum.tile([H, RPC], f32, tag="ps", padded_shape=[128, 1024])
                    nc.tensor.matmul(agp[:], lhsT=w2_sb[:], rhs=st["sst"][:], start=True, stop=True)
                    nc.vector.tensor_add(combt[0:H, :], agp[:], b2t_sb[:])

                def s2():
                    h2p = psum.tile([H, RPC], f32, tag="ps", padded_shape=[128, 1024])
                    nc.tensor.matmul(h2p[:], lhsT=u1_sb[:], rhs=combt[:], start=True, stop=True)
                    r1 = scr.tile([H, RPC], f32, tag="r1")
                    nc.scalar.activation(r1[:], h2p[:], relu, bias=ub1_sb[:, 0:1])
                    st["r1"] = r1

                def s3():
                    o2p = psum.tile([H, RPC], f32, tag="ps", padded_shape=[128, 1024])
                    nc.tensor.matmul(o2p[:], lhsT=u2_sb[:], rhs=st["r1"][:], start=True, stop=True)
                    o2 = scr.tile([H, RPC], f32, tag="o2")
                    nc.vector.tensor_scalar_add(o2[:], o2p[:], ub2_sb[:, 0:1])
                    st["o2"] = o2

                def s4():
                    fin = psum.tile([RPC, H], f32, tag="ps", padded_shape=[128, 1024])
                    nc.tensor.transpose(fin[:], st["o2"][:], iden_sb[:])
                    osb = scr.tile([RPC, H], f32, tag="osb")
                    nc.scalar.activation(osb[:], fin[:], mybir.ActivationFunctionType.Copy)
                    nc.sync.dma_start(out[:], osb[:])

                return [s0, s1, s2, s3, s4]

            # interleave positions: epilogue stage k of the previous phase is
            # emitted before pair 8k of the current phase, giving each
            # cross-engine dependency ~8 pairs (~3us) of slack.
            EP_AT = {0: 0, 8: 1, 16: 2, 24: 3, 32: 4}

            # processing order: optionally alternate ACT/DVE pairs evenly so
            # the shared psum rotation never sees long same-engine runs
            if INTERLEAVE:
                Al = [p for p in range(NPAIR) if isact[p]]
                Dl = [p for p in range(NPAIR) if not isact[p]]
                keyed = [((i + 0.5) / len(Al), p) for i, p in enumerate(Al)]
                keyed += [((i + 0.5) / len(Dl), p) for i, p in enumerate(Dl)]
                PROC = [p for _, p in sorted(keyed)]
            else:
                PROC = list(range(NPAIR))

            def _pairs(ph, interleave=None):
                aa, ad = accs[ph]
                for idx, p in enumerate(PROC):
                    if interleave is not None and idx in EP_AT:
                        interleave[EP_AT[idx]]()
                    q = int(pair_chunk[p])
                    lo = int(offs[p] - offs[CHUNK_B[q]])
                    jp = J[p]
                    ps = psum.tile([128, 1024], f32, tag="ps")
                    c0 = min(jp, 512)
                    nc.tensor.matmul(
                        ps[:, 0:c0],
                        lhsT=statw_sb[:],
                        rhs=pk[q][:, lo : lo + c0],
                        start=True,
                        stop=True,
                    )
                    if jp > 512:
                        nc.tensor.matmul(
                            ps[:, 512:jp],
                            lhsT=statw_sb[:],
                            rhs=pk[q][:, lo + 512 : lo + jp],
                            start=True,
                            stop=True,
                        )
                    if isact[p]:
                        o = ps if INPLACE else scrA.tile([128, 1024], f32, tag="oA")
                        nc.scalar.activation(
                            o[:, 0:jp],
                            ps[:, 0:jp],
                            relu,
                            bias=abias_sb[:, p : p + 1],
                            accum_out=aa[:, p : p + 1],
                        )
                    else:
                        o = ps if INPLACE else scrD.tile([128, 1024], f32, tag="oD")
                        nc.vector.tensor_scalar(
                            o[:, 0:jp],
                            ps[:, 0:jp],
                            nabias_sb[:, p : p + 1],
                            0.0,
                            op0=mybir.AluOpType.max,
                            op1=mybir.AluOpType.add,
                            accum_out=ad[:, p : p + 1],
                        )

            if reps == 1:
                _pairs(0)
                for s in _epilogue_stages(0):
                    s()
            else:
                assert reps % UNROLL == 0, "looped variant needs reps % UNROLL == 0"
                with tc.For_i(0, reps // UNROLL, 1, staggered_reset=STAGGER):
                    for u in range(UNROLL):
                        ph = u % 2
                        _pairs(ph, interleave=_epilogue_stages(1 - ph))
                for s in _epilogue_stages(1):
                    s()

    nc.compile()
    return nc


def _prep_maps(node_features, edge_features, adjacency, W1, b1, W2, b2, U1, ub1, U2, ub2):
    nf = np.ascontiguousarray(node_features, np.float32)
    ef = np.ascontiguousarray(edge_features, np.float32)
    adj = np.asarray(adjacency)
    W1 = np.asarray(W1, np.float32)
    b1 = np.asarray(b1, np.float32)
    W2f = np.asarray(W2, np.float32)

    adj_key = hash(adj.tobytes())
    if _cache.get("sched_adj_key") != adj_key:
        Jt, isact, perms = _mk_schedule(adj)
        _cache["sched"] = (Jt, isact)
        _cache["perms"] = perms
        _cache["sched_adj_key"] = adj_key
    Jt, isact = _cache["sched"]
    perms = _cache["perms"]
    offs = np.zeros(NPAIR + 1, np.int64)
    offs[1:] = np.cumsum(Jt)
    TOT = int(offs[-1])
    isact_f = np.array(isact, bool)

    W1i, W1j, W1e = W1[0:D], W1[D : 2 * D], W1[2 * D :]
    A = nf @ W1i + b1[None, :]              # (N, H) fp32, exact bias term
    mask = adj > 0
    deg = adj.sum(axis=1).astype(np.float32)
    cnt = mask.sum(axis=1)
    degc = np.where(deg == 0, 1.0, deg)

    stat = np.zeros((KTOT, 128), np.float32)
    stat[0:16, 0:64] = W1e
    stat[16:32, 64:128] = W1e
    stat[32:64, 0:64] = W1j
    stat[64:96, 64:128] = W1j
    stat_bf = stat.astype(BF16)

    nfT_q = nf.T.astype(PACK_NP)            # (32, 1024)
    ef3 = ef.reshape(N, N, E)

    maps = []
    for core in range(N_CORES):
        i0 = core * RPC
        perm = perms[core]
        ia = perm[0::2] + i0                # global row idx of pair-lo
        ib = perm[1::2] + i0
        pack = np.zeros((KTOT, TOT), PACK_NP)
        for p in range(NPAIR):
            off = int(offs[p])
            ja = np.nonzero(mask[ia[p]])[0]
            jb = np.nonzero(mask[ib[p]])[0]
            pack[0:16, off : off + len(ja)] = ef3[ia[p], ja].T.astype(PACK_NP)
            pack[16:32, off : off + len(jb)] = ef3[ib[p], jb].T.astype(PACK_NP)
            pack[32:64, off : off + len(ja)] = nfT_q[:, ja]
            pack[64:96, off : off + len(jb)] = nfT_q[:, jb]

        Aa = A[ia]                          # (NPAIR, H)
        Ab = A[ib]
        abias_c = np.concatenate([Aa.T, Ab.T], axis=0)       # (128, NPAIR)
        cnt_c = np.concatenate(
            [np.broadcast_to(cnt[ia][None, :], (H, NPAIR)),
             np.broadcast_to(cnt[ib][None, :], (H, NPAIR))], axis=0
        ).astype(np.float32)
        Jrow = np.broadcast_to(np.array(Jt, np.float32)[None, :], (128, NPAIR))
        npad = Jrow - cnt_c
        relu_a = np.maximum(abias_c, 0.0)
        relu_na = np.maximum(-abias_c, 0.0)
        # ACT pairs: acc = sum_real relu(h+a) + npad*relu(a)
        # DVE pairs: acc = sum_real relu(h+a) - cnt*a + npad*relu(-a)
        fixup_c = np.where(
            isact_f[None, :],
            -npad * relu_a,
            cnt_c * abias_c - npad * relu_na,
        ).astype(np.float32)

        rd_a = (1.0 / degc[ia]).astype(np.float32)           # (NPAIR,)
        rd_b = (1.0 / degc[ib]).astype(np.float32)
        rda64 = np.broadcast_to(rd_a[None, :], (H, NPAIR)).astype(np.float32)
        rdb64 = np.broadcast_to(rd_b[None, :], (H, NPAIR)).astype(np.float32)

        # fold (fixup * rdeg) @ W2 into the b2 term:
        # F64[h', s] = fixup[(s%2)*64+h', s//2] * rdeg_s[s]
        F64 = np.empty((H, RPC), np.float32)
        F64[:, 0::2] = fixup_c[0:H, :] * rd_a[None, :]
        F64[:, 1::2] = fixup_c[H:128, :] * rd_b[None, :]
        fix_term = W2f.T @ F64                                # (H, RPC)

        iseq = np.empty(RPC, np.int64)
        iseq[0::2] = ia
        iseq[1::2] = ib
        b2t_c = (
            np.asarray(b2, np.float32)[:, None]
            * (cnt[iseq] / degc[iseq])[None, :]
            + fix_term
        ).astype(np.float32)

        rd128 = np.concatenate([rda64, rdb64], axis=0)       # (128, NPAIR)
        c128 = np.concatenate([abias_c, -abias_c, rd128], axis=1).astype(np.float32)
        c64 = np.concatenate(
            [
                np.ascontiguousarray(b2t_c, np.float32),
                W2f,
                np.asarray(U2, np.float32),
                np.eye(H, dtype=np.float32),
                np.asarray(ub1, np.float32).reshape(H, 1),
                np.asarray(ub2, np.float32).reshape(H, 1),
            ],
            axis=1,
        ).astype(np.float32)

        m = {
            "statw": stat_bf,
            "c128": np.ascontiguousarray(c128),
            "c64": np.ascontiguousarray(c64),
            "u1m": np.concatenate(
                [np.asarray(U1, np.float32)[D:], np.asarray(U1, np.float32)[:D]]
            ),
            "xct": np.ascontiguousarray(nf[iseq].T, np.float32),
        }
        for q in range(NCHUNK):
            a0 = int(offs[CHUNK_B[q]])
            a1 = int(offs[CHUNK_B[q + 1]])
            m[f"pack{q}"] = np.ascontiguousarray(pack[:, a0:a1])
        maps.append(m)
    return maps


def kernel(**inputs) -> np.ndarray:
    maps = _prep_maps(
        inputs["node_features"],
        inputs["edge_features"],
        inputs["adjacency"],
        inputs["W1"],
        inputs["b1"],
        inputs["W2"],
        inputs["b2"],
        inputs["U1"],
        inputs["ub1"],
        inputs["U2"],
        inputs["ub2"],
    )
    key = ("nc", PACK_FP8, ACT_FIXED, SCR_BUFS, UNROLL, STAGGER, INTERLEAVE, INPLACE, _cache["sched"])
    if key not in _cache:
        _cache[key] = _build()
    nc = _cache[key]
    res = run_bass_kernel_spmd(nc, maps, list(range(N_CORES)))
    full = np.empty((N, H), np.float32)
    for c in range(N_CORES):
        o = np.asarray(res.results[c]["out"], np.float32)
        full[c * RPC + _cache["perms"][c]] = o
    return full
